# revision 11
# baseline (speedup 1.0000x reference)
"""GCNConv kernel for 8 Trainium2 NeuronCores.

Math: out = CSR_neighbor_sum(X @ W) == (CSR_neighbor_sum(X)) @ W
(the unweighted neighbor sum commutes with the right-multiplication by W),
so each core gathers+sums raw X rows and applies the small [128,128] weight
matmul afterwards.

Strategy (hardcoded for N=100000 nodes, degree<=16, D=128, 8 cores):
  - Output nodes sharded across 8 cores (12500 rows each); X, W replicated.
  - Neighbor rows are fetched with the batched SWDGE `dma_gather` custom
    instruction (bf16 table, 256B rows, int16 indices) spread across all
    4 SWDGE queues (4 Q7 core pairs run concurrently); measured ~1ns per
    gathered row at >=1536 indices/instruction vs ~6ns/row for the classic
    one-row-per-partition indirect-DMA path.
  - int16 indices only address 32K rows, so X is split into 4 windows of
    25088 rows (a zero row is appended to each window for padding). Tiles
    of 128 nodes are processed in PAIRS: one dma_gather per (pair, window)
    keeps indices/instruction high. Slots are node-major ([128 partitions =
    nodes, chunks]) with compile-time per-tile widths M[t][w] = max window
    count over the tile's nodes (and over cores, to stay SPMD). Nodes are
    sorted per core by window-count vector so tiles are homogeneous (total
    pad ~1.5x edges instead of ~2.2x unsorted); the host un-permutes the
    output rows at the end.
  - A strided DVE binary tree sums each tile's window chunks (bf16
    intermediates, fp32 combine), PE transposes the sum and multiplies by
    W (fp32), and the result is stored to the core's output shard.

The NEFF is compiled per edge-structure signature (the M widths); for a
given graph the kernel compiles once and is cached.
"""

import time

import numpy as np

N_NODES = 100000
DEG = 16
D = 128
N_CORES = 8
NODES_PER_CORE = N_NODES // N_CORES  # 12500
P = 128  # SBUF partitions / nodes per tile
N_TILES = (NODES_PER_CORE + P - 1) // P  # 98
SENTINEL = N_NODES  # "no edge" marker in the host edge matrix
NW = 4  # index windows (int16 limit)
WIN = 25088  # rows per window (4 * 25088 >= 100000)
WROWS = WIN + 1  # +1 zero pad row per window in the gather table
MIN_CHUNKS = 16  # target >=2048 idxs per gather instruction (16 chunks)
MAX_GTILES = 16  # cap tiles per window-group (staging liveness bound)

_CACHE = {}


def _wgroups(m_widths):
    """Per-window tile grouping: consecutive tiles are grouped until the
    window's chunk total reaches MIN_CHUNKS. Returns, per window, a dict
    start_tile -> list of (tile, m, chunk_offset) for that group."""
    m = np.asarray(m_widths, np.int32)
    out = []
    for w in range(NW):
        groups = {}
        t = 0
        while t < N_TILES:
            members = []
            tot = 0
            start = t
            while t < N_TILES and (tot < MIN_CHUNKS or not members) \
                    and len(members) < MAX_GTILES:
                if m[t, w] > 0:
                    members.append((t, int(m[t, w]), tot))
                    tot += int(m[t, w])
                t += 1
                if tot >= MIN_CHUNKS:
                    break
            if members:
                groups[start] = members
            else:
                break
        out.append(groups)
    return out


def _build_nc(m_widths):
    """Construct and compile the SPMD per-core Bass program. `m_widths` is
    an [N_TILES][NW] tuple of per-tile per-window chunk counts."""
    import concourse.mybir as mybir
    from concourse import bacc
    from concourse.tile import TileContext
    from concourse.masks import make_identity

    m_widths = np.asarray(m_widths, np.int32)
    tail = NODES_PER_CORE - (N_TILES - 1) * P  # 84
    wgroups = _wgroups(m_widths)
    idx_cols = 8 * int(m_widths.sum())

    nc = bacc.Bacc("TRN2", target_bir_lowering=False, debug=False,
                   enable_asserts=True, num_devices=N_CORES,
                   dynamic_dma_scratch_size=65536,
                   num_swdge_queues=NW)
    XT = nc.dram_tensor("XT", [NW * WROWS, D], mybir.dt.bfloat16,
                        kind="ExternalInput")
    W = nc.dram_tensor("W", [D, D], mybir.dt.float32, kind="ExternalInput")
    idx = nc.dram_tensor("idx", [P, idx_cols], mybir.dt.int16,
                         kind="ExternalInput")
    out = nc.dram_tensor("out", [NODES_PER_CORE, D], mybir.dt.float32,
                         kind="ExternalOutput")

    with TileContext(nc) as tc:
        with (
            tc.tile_pool(name="const", bufs=1) as cpool,
            tc.tile_pool(name="g0", bufs=2) as gp0,
            tc.tile_pool(name="g1", bufs=2) as gp1,
            tc.tile_pool(name="g2", bufs=2) as gp2,
            tc.tile_pool(name="g3", bufs=2) as gp3,
            tc.tile_pool(name="yp", bufs=3) as ypool,
            tc.tile_pool(name="op", bufs=3) as opool,
            tc.tile_pool(name="ps", bufs=4, space="PSUM") as pspool,
        ):
            gpools = [gp0, gp1, gp2, gp3]
            w_sb = cpool.tile([D, D], mybir.dt.float32)
            nc.sync.dma_start(out=w_sb[:], in_=W[:])
            ident = cpool.tile([P, P], mybir.dt.float32)
            make_identity(nc, ident[:])
            idx_sb = cpool.tile([P, idx_cols], mybir.dt.int16)
            nc.sync.dma_start(out=idx_sb[:], in_=idx[:])

            col = 0
            # active staging tile + per-tile (m, chunk offset) per window
            active = [None] * NW
            for t in range(N_TILES):
                nt = P if t < N_TILES - 1 else tail
                r0 = t * P
                for w in range(NW):
                    members = wgroups[w].get(t)
                    if members is None:
                        continue
                    chunks = sum(m for _, m, _ in members)
                    g = gpools[w].tile([P, chunks, D], mybir.dt.bfloat16,
                                       tag=f"g{w}")
                    ni = chunks * P
                    nc.gpsimd.dma_gather(
                        g[:, :, :],
                        XT[w * WROWS:(w + 1) * WROWS, :],
                        idx_sb[:, col:col + chunks * 8],
                        ni, ni, D, queue_num=w, single_packet=False)
                    col += chunks * 8
                    active[w] = (g, {tt: (m, off) for tt, m, off in members})

                parts = []
                for w in range(NW):
                    if active[w] is None or t not in active[w][1]:
                        continue
                    g, mem = active[w]
                    m, off = mem[t]
                    while m > 1:
                        h = m // 2
                        nc.vector.tensor_add(
                            out=g[:, off:off + h, :],
                            in0=g[:, off:off + h, :],
                            in1=g[:, off + m - h:off + m, :],
                        )
                        m -= h
                    parts.append((g, off))

                y = ypool.tile([P, D], mybir.dt.float32, tag="y")
                if True:
                    if len(parts) == 1:
                        g, off = parts[0]
                        nc.vector.tensor_copy(out=y[:, :], in_=g[:, off, :])
                    else:
                        (g0_, o0), (g1_, o1) = parts[0], parts[1]
                        nc.vector.tensor_add(out=y[:, :], in0=g0_[:, o0, :],
                                             in1=g1_[:, o1, :])
                        for g, off in parts[2:]:
                            nc.vector.tensor_add(out=y[:, :], in0=y[:, :],
                                                 in1=g[:, off, :])

                    yt_ps = pspool.tile([D, P], mybir.dt.float32, tag="yt")
                    nc.tensor.transpose(out=yt_ps[:, :nt], in_=y[:nt, :],
                                        identity=ident[:nt, :nt])
                    yt_sb = ypool.tile([D, P], mybir.dt.float32, tag="yt_sb")
                    nc.vector.tensor_copy(out=yt_sb[:, :nt],
                                          in_=yt_ps[:, :nt])

                    o_ps = pspool.tile([P, D], mybir.dt.float32, tag="ops")
                    nc.tensor.matmul(out=o_ps[:nt, :], lhsT=yt_sb[:, :nt],
                                     rhs=w_sb[:], start=True, stop=True)
                    o_sb = opool.tile([P, D], mybir.dt.float32, tag="osb")
                    nc.vector.tensor_copy(out=o_sb[:nt, :], in_=o_ps[:nt, :])
                    nc.sync.dma_start(out=out[r0:r0 + nt, :],
                                      in_=o_sb[:nt, :])
    nc.compile()
    return nc


def _get_nc(m_widths):
    key = tuple(tuple(int(v) for v in mw) for mw in m_widths)
    if key not in _CACHE:
        _CACHE[key] = _build_nc(key)
    return _CACHE[key]


def _edge_matrix(row_pointers, column_index):
    """Per-node [N_NODES, w_pad] int32 neighbor matrix from the CSR arrays,
    padded with SENTINEL. Fast path for uniform degree DEG."""
    rp = np.asarray(row_pointers).astype(np.int64)
    ci = np.asarray(column_index).astype(np.int32)
    deg = np.diff(rp)
    if len(deg) == N_NODES and (deg == DEG).all() and rp[0] == 0 \
            and rp[-1] == len(ci):
        return ci.reshape(N_NODES, DEG), DEG
    e = np.arange(len(ci), dtype=np.int64)
    rows = np.searchsorted(rp, e, side="right") - 1
    valid = (rows >= 0) & (rows < N_NODES)
    rows = rows[valid]
    cols = ci[valid]
    order = np.argsort(rows, kind="stable")
    rows, cols = rows[order], cols[order]
    counts = np.bincount(rows, minlength=N_NODES)
    w_pad = max(int(counts.max()) if len(counts) else 1, 1)
    mat = np.full((N_NODES, w_pad), SENTINEL, dtype=np.int32)
    starts = np.zeros(N_NODES + 1, dtype=np.int64)
    np.cumsum(counts, out=starts[1:])
    slot = np.arange(len(rows)) - starts[rows]
    mat[rows, slot] = np.clip(cols, 0, N_NODES - 1)
    return mat, w_pad


def _plan(edges):
    """Host planning: per-core node ordering + per-tile window widths."""
    wofe = np.minimum(edges // WIN, NW - 1)
    wofe[edges >= N_NODES] = -1
    counts = np.zeros((N_NODES, NW), np.int32)
    for w in range(NW):
        counts[:, w] = (wofe == w).sum(1)

    orders = np.full((N_CORES, N_TILES * P), -1, np.int64)
    m_per_core = np.zeros((N_CORES, N_TILES, NW), np.int32)
    for c in range(N_CORES):
        lo = c * NODES_PER_CORE
        cc = counts[lo:lo + NODES_PER_CORE]
        o = np.lexsort((cc[:, 3], cc[:, 2], cc[:, 1], cc[:, 0]))
        orders[c, :NODES_PER_CORE] = lo + o
        srt = cc[o]
        full = N_TILES - 1
        m_per_core[c, :full] = srt[:full * P].reshape(full, P, NW).max(1)
        m_per_core[c, full] = srt[full * P:].max(0)
    m_widths = m_per_core.max(0)
    return orders, m_widths, counts


def _tile_block(edges, wofe, nodes, w, m):
    """int16 index block [16, m*8] for one (tile, window)."""
    valid = nodes >= 0
    nb = edges[np.clip(nodes, 0, N_NODES - 1)]
    nw = wofe[np.clip(nodes, 0, N_NODES - 1)].copy()
    nw[~valid] = -1
    blk = np.full((P, m), WIN, np.int16)
    sel = nw == w
    cnt = sel.sum(1)
    for p in np.nonzero(cnt)[0]:
        vals = nb[p][sel[p]] - w * WIN
        blk[p, :len(vals)] = vals.astype(np.int16)
    ib = blk.T.reshape(m * P)  # position-ordered (i = j*128 + p)
    return ib.reshape(m * 8, 16).T


def _build_idx(edges, orders, m_widths, core):
    """Per-core int16 index array [P, idx_cols] in dma_gather layout.
    Block order must match _build_nc's traversal: for each tile t, for each
    window w whose group starts at t, the group's member-tile blocks."""
    m_widths = np.asarray(m_widths, np.int32)
    wgroups = _wgroups(m_widths)
    idx_cols = 8 * int(m_widths.sum())
    out = np.empty((16, idx_cols), np.int16)
    wofe = np.minimum(edges // WIN, NW - 1)
    wofe[edges >= N_NODES] = -1
    order = orders[core]
    col = 0
    for t in range(N_TILES):
        for w in range(NW):
            members = wgroups[w].get(t)
            if members is None:
                continue
            for tt, m, _off in members:
                nodes = order[tt * P:(tt + 1) * P]
                out[:, col:col + m * 8] = _tile_block(
                    edges, wofe, nodes, w, m)
                col += m * 8
    assert col == idx_cols, (col, idx_cols)
    return np.tile(out, (8, 1))


def _make_xt(X):
    """bf16 gather table: 4 windows of WIN rows, each + 1 zero row."""
    import ml_dtypes
    xt = np.zeros((NW * WROWS, D), dtype=ml_dtypes.bfloat16)
    Xb = X.astype(ml_dtypes.bfloat16)
    for w in range(NW):
        lo = w * WIN
        hi = min(lo + WIN, N_NODES)
        if hi > lo:
            xt[w * WROWS:w * WROWS + (hi - lo)] = Xb[lo:hi]
    return xt


def kernel(X, weights, row_pointers, column_index, blockPartition,
           edgeToColumn, edgeToRow):
    from concourse.bass_utils import run_bass_kernel_spmd

    X = np.asarray(X, dtype=np.float32)
    W = np.ascontiguousarray(np.asarray(weights), dtype=np.float32)
    edges, w_pad = _edge_matrix(row_pointers, column_index)
    orders, m_widths, counts = _plan(edges)
    xt = _make_xt(X)

    nc = _get_nc(m_widths)
    in_maps = []
    for c in range(N_CORES):
        in_maps.append({
            "XT": xt,
            "W": W,
            "idx": _build_idx(edges, orders, m_widths, c),
        })
    last_exc = None
    for _attempt in range(3):
        try:
            res = run_bass_kernel_spmd(nc, in_maps,
                                       core_ids=list(range(N_CORES)))
            break
        except Exception as exc:  # transient NRT/axon errors recover on retry
            last_exc = exc
            time.sleep(15)
    else:
        raise last_exc
    out = np.empty((N_NODES, D), np.float32)
    for c in range(N_CORES):
        rows = res.results[c]["out"]
        out[orders[c, :NODES_PER_CORE]] = rows
    return out
